# revision 14
# baseline (speedup 1.0000x reference)
"""AutoRound/GPTQ int4 linear on 8 Trainium2 NeuronCores.

y = x @ dequant(qweight, qzeros, scales), computed in bf16 like the torch
module: deq = (w_int4 - zeros[g]) * scales[g] in fp32, cast to bf16;
y = bf16_matmul(x.bf16, deq.bf16) with fp32 accumulation, output cast
back to fp32.

Sharding: 8 cores = 4-way tensor-parallel on out_features (1024 each)
x 2-way data-parallel on tokens (4096 each). Each core dequantizes its
weight slice on-chip and computes [1024 out, 4096 tok] bf16; the host
reassembles.

Per-core schedule (PE roofline here is 2048 matmuls x 216 ns = 442 us):
- x is cast to bf16 on the host (RNE, identical to the reference's
  astype) and fed through plain HWDGE DMAs on the sync ring; the old
  SWDGE fp32->bf16 converting-DMA path kept gpsimd busy ~172 us and
  doubled x HBM traffic.
- The contraction (in_features) index is interleaved so that SBUF
  k-chunk `cc = blk*8 + j` holds k = blk*1024 + 8*p + j at partition p.
  Nibble j of packed qweight row p is the weight for partition p of
  chunk cc, so the int4 unpack is one fused shift+mask tensor_scalar
  per chunk with a *constant* shift; qweight is host-split into int16
  low/high planes so the dequant chain runs in 16-bit DVE fast modes.
  zeros/scales are host-unpacked/replicated x16 (tiny metadata).
- Token tile 0 runs chunk-OUTER across all 8 PSUM banks: each dequanted
  chunk is consumed by 8 matmuls (1.7 us) while the DVE produces the
  next chunk in 0.8 us, so the PE never waits for the dequant frontier
  (the old os-outer order stalled ~11 us and re-tripped the HAM
  clock-gate to half rate for 10 us). Tiles 1..7 run os-outer with one
  PSUM bank open at a time.
- All PSUM->SBUF copies run on the SCALAR engine: DVE CAST reads of
  PSUM were observed to stall concurrent matmuls ~432 ns each.
- Block-0 weight DMAs go out in parallel on sync/scalar/gpsimd queues;
  tile-0 x arrives as 32 per-chunk DMAs interleaved with the qwl block
  loads so the first matmul can start ~6.5 us in. A short N=128 dummy-
  matmul warmup fills the preamble window and trips the HAM clock-gate
  to 2.4 GHz before the real stream starts.
- y is staged per token tile in one [128, 8*512] SBUF tile and written
  with a single strided DMA per tile (the last tile uses per-os writes
  to keep the tail short); fewer DMA completions also shorten the
  epilogue semaphore drain.
"""

import numpy as np
import ml_dtypes

PACK = 8
IN_F = 4096
OUT_F = 4096
GROUP = 128
B, S = 4, 2048
T_TOTAL = B * S  # 8192

N_CORES = 8
TP = 4  # out_feature shards
DP = 2  # token shards
NO = OUT_F // TP  # 1024 out features per core
TP_T = T_TOTAL // DP  # 4096 tokens per core
NT = 512  # token tile (matmul moving free dim / one PSUM bank)
NTILE = TP_T // NT  # 8
KB = IN_F // 1024  # 4 k-blocks of 1024 (8 chunks of 128 each) = x quarters
WARMUP_MM = 46


def build_nc(no=NO, t=TP_T, nt=NT, kblocks=KB):
    import concourse.bacc as bacc
    import concourse.mybir as mybir
    from concourse.tile import TileContext

    dt = mybir.dt
    alu = mybir.AluOpType
    n_chunks = kblocks * 8
    n_os = no // 128
    ntile = t // nt

    nc = bacc.Bacc("TRN2", target_bir_lowering=False, debug=False)

    # x^T, k-interleaved and tiled: row (tt*KB + q)*128 + p, col j*nt + c
    # holds x[token tt*nt + c, k = q*1024 + 8p + j] as bf16.
    xt_d = nc.dram_tensor(
        "xt", [ntile * kblocks * 128, 8 * nt], dt.bfloat16, kind="ExternalInput"
    )
    # low/high int16 halves of the packed int32 qweight (host-split):
    # nibbles j=0..3 live in the low half, j=4..7 in the high half.
    qwl_d = nc.dram_tensor("qwl", [kblocks * 128, no], dt.int16, kind="ExternalInput")
    qwh_d = nc.dram_tensor("qwh", [kblocks * 128, no], dt.int16, kind="ExternalInput")
    # zeros (host-unpacked int16) and scales, group rows pre-replicated x16
    zf_d = nc.dram_tensor("zf", [kblocks * 128, no], dt.int16, kind="ExternalInput")
    sc_d = nc.dram_tensor("sc", [kblocks * 128, no], dt.float16, kind="ExternalInput")
    # block 0 pre-dequantized on host (bit-identical bf16): the matmul
    # stream's start then gates on a single 0.25MB DMA instead of the
    # qwl0+zf0+sc0 fan-in plus a 2us DVE chain, all of which run at ~1/3
    # rate this early in the kernel. Blocks 1-3 still dequant on device.
    wd0_d = nc.dram_tensor("wd0", [8 * 128, no], dt.bfloat16, kind="ExternalInput")
    # y[p, os, tok]: out feature os*128 + p
    y_d = nc.dram_tensor("y", [128, n_os, t], dt.bfloat16, kind="ExternalOutput")

    with TileContext(nc) as tc:
        with (
            tc.tile_pool(name="wd", bufs=1) as wd_pool,
            tc.tile_pool(name="qw", bufs=2) as qw_pool,
            tc.tile_pool(name="sbc", bufs=2) as sbc_pool,
            tc.tile_pool(name="zf", bufs=2) as zf_pool,
            tc.tile_pool(name="wi", bufs=5) as wi_pool,
            tc.tile_pool(name="xq", bufs=2) as xq_pool,
            tc.tile_pool(name="ps", bufs=8, space="PSUM") as ps_pool,
            tc.tile_pool(name="yo", bufs=2) as yo_pool,
            tc.tile_pool(name="yol", bufs=4) as yol_pool,
        ):
            # memset first so the PE warmup can start during DMA issue
            warm = qw_pool.tile([128, 128], dt.bfloat16, tag="warm")
            nc.vector.memset(warm[:], 0.0)

            qw_sbs = []
            zf_tiles = [None] * kblocks
            sbc_tiles = [None] * kblocks

            def load_block(blk):
                # blocks 1-3 only (block 0 arrives pre-dequantized): the
                # qw planes ride the scalar HWDGE ring, zeros/scales ride
                # SWDGE — none of these gate the matmul stream start.
                qwl_sb = qw_pool.tile([128, no], dt.int16, tag=f"qwl{blk % 2}")
                qwh_sb = qw_pool.tile([128, no], dt.int16, tag=f"qwh{blk % 2}")
                qw_sbs.append((qwl_sb, qwh_sb))
                zf = zf_pool.tile(
                    [128, no], dt.int16, tag=f"zf{blk % 2}", name=f"zf{blk}"
                )
                sbc = sbc_pool.tile([128, no], dt.float16, tag=f"sbc{blk % 2}")
                nc.scalar.dma_start(
                    out=qwl_sb[:], in_=qwl_d[blk * 128 : (blk + 1) * 128, :]
                )
                nc.scalar.dma_start(
                    out=qwh_sb[:], in_=qwh_d[blk * 128 : (blk + 1) * 128, :]
                )
                nc.gpsimd.dma_start(out=zf[:], in_=zf_d[blk * 128 : (blk + 1) * 128, :])
                nc.gpsimd.dma_start(
                    out=sbc[:], in_=sc_d[blk * 128 : (blk + 1) * 128, :]
                )
                zf_tiles[blk] = zf
                sbc_tiles[blk] = sbc

            # PE warmup: small-N dummy matmuls fill the pre-data window and
            # trip the HAM clock gate before the real stream starts.
            ps_w = ps_pool.tile([128, nt], dt.float32, tag="ps")
            for _ in range(WARMUP_MM):
                nc.tensor.matmul(
                    out=ps_w[:, 0:128],
                    lhsT=warm[:],
                    rhs=warm[:],
                    start=True,
                    stop=True,
                )

            wd_tiles = [None] * n_chunks
            # stream-critical loads, back-to-back on the sync ring: the
            # pre-dequantized block-0 chunks interleaved with tile-0's
            # quarter-0 x chunks, in exactly consumption order.
            xq_tiles = {}
            xb0 = xq_pool.tile([128, 8 * nt], dt.bfloat16, tag="xq0", name="x0q0")
            for j in range(8):
                wdc = wd_pool.tile([128, no], dt.bfloat16, tag=f"wd{j}")
                nc.sync.dma_start(out=wdc[:], in_=wd0_d[j * 128 : (j + 1) * 128, :])
                wd_tiles[j] = wdc
                nc.sync.dma_start(
                    out=xb0[:, j * nt : (j + 1) * nt],
                    in_=xt_d[0:128, j * nt : (j + 1) * nt],
                )
            t0 = [xb0]
            for blk in range(1, kblocks):
                load_block(blk)
            # tile-0 quarters 1-3 go on the scalar ring (sync stays clear
            # for the stream-critical chunk loads above)
            for q in range(1, kblocks):
                xb = xq_pool.tile(
                    [128, 8 * nt], dt.bfloat16, tag=f"xq{q}", name=f"x0q{q}"
                )
                nc.scalar.dma_start(out=xb[:], in_=xt_d[q * 128 : (q + 1) * 128, :])
                t0.append(xb)
            xq_tiles[0] = t0

            def load_x(tt):
                tiles = []
                for q in range(kblocks):
                    xb = xq_pool.tile(
                        [128, 8 * nt], dt.bfloat16, tag=f"xq{q}", name=f"x{tt}q{q}"
                    )
                    nc.sync.dma_start(
                        out=xb[:],
                        in_=xt_d[(tt * kblocks + q) * 128 : (tt * kblocks + q + 1) * 128, :],
                    )
                    tiles.append(xb)
                xq_tiles[tt] = tiles

            # ---- dequantize blocks 1-3 into per-chunk tiles [128, no]
            for blk in range(1, kblocks):
                qwl_sb, qwh_sb = qw_sbs[blk - 1]
                for j in range(8):
                    cc = blk * 8 + j
                    wi = wi_pool.tile([128, no], dt.int16, tag="wi_i")
                    nc.vector.tensor_scalar(
                        out=wi[:],
                        in0=(qwl_sb if j < 4 else qwh_sb)[:],
                        scalar1=4 * (j % 4),
                        scalar2=15,
                        op0=alu.logical_shift_right,
                        op1=alu.bitwise_and,
                    )
                    wb = wi_pool.tile([128, no], dt.bfloat16, tag="wi_b")
                    nc.vector.tensor_sub(out=wb[:], in0=wi[:], in1=zf_tiles[blk][:])
                    wdc = wd_pool.tile([128, no], dt.bfloat16, tag=f"wd{cc}")
                    nc.vector.tensor_mul(out=wdc[:], in0=wb[:], in1=sbc_tiles[blk][:])
                    wd_tiles[cc] = wdc

            load_x(1)

            # ---- token tile 0: chunk-outer so the PE chases the dequant
            # frontier with zero stalls (8 PSUM banks accumulate at once).
            ps_t0 = [
                ps_pool.tile([128, nt], dt.float32, tag="ps", name=f"ps0_{o}")
                for o in range(n_os)
            ]
            for k in range(n_chunks):
                rhs = xq_tiles[0][k // 8][:, (k % 8) * nt : (k % 8 + 1) * nt]
                for os_ in range(n_os):
                    nc.tensor.matmul(
                        out=ps_t0[os_][:],
                        lhsT=wd_tiles[k][:, os_ * 128 : (os_ + 1) * 128],
                        rhs=rhs,
                        start=(k == 0),
                        stop=(k == n_chunks - 1),
                    )
            yo = yo_pool.tile([128, n_os * nt], dt.bfloat16, tag="yo", name="yo0")
            for os_ in range(n_os):
                nc.scalar.copy(out=yo[:, os_ * nt : (os_ + 1) * nt], in_=ps_t0[os_][:])
            nc.scalar.dma_start(out=y_d[:, :, 0:nt], in_=yo[:])

            # ---- token tiles 1..: os-outer, one PSUM bank open at a time
            for tt in range(1, ntile):
                if tt + 1 < ntile:
                    load_x(tt + 1)
                xts = xq_tiles.pop(tt)
                last = tt == ntile - 1
                yo = (
                    None
                    if last
                    else yo_pool.tile(
                        [128, n_os * nt], dt.bfloat16, tag="yo", name=f"yo{tt}"
                    )
                )
                for os_ in range(n_os):
                    ps = ps_pool.tile([128, nt], dt.float32, tag="ps", name="ps")
                    for k in range(n_chunks):
                        nc.tensor.matmul(
                            out=ps[:],
                            lhsT=wd_tiles[k][:, os_ * 128 : (os_ + 1) * 128],
                            rhs=xts[k // 8][:, (k % 8) * nt : (k % 8 + 1) * nt],
                            start=(k == 0),
                            stop=(k == n_chunks - 1),
                        )
                    if last:
                        # per-os write-out keeps the final copy+DMA short;
                        # the very last group splits across scalar+vector
                        # and two DMA rings to halve the tail
                        yol = yol_pool.tile(
                            [128, nt], dt.bfloat16, tag="yol", name="yol"
                        )
                        if os_ == n_os - 1:
                            h = nt // 2
                            nc.scalar.copy(out=yol[:, 0:h], in_=ps[:, 0:h])
                            nc.vector.tensor_copy(out=yol[:, h:nt], in_=ps[:, h:nt])
                            nc.scalar.dma_start(
                                out=y_d[:, os_ : os_ + 1, tt * nt : tt * nt + h],
                                in_=yol[:, 0:h],
                            )
                            nc.sync.dma_start(
                                out=y_d[:, os_ : os_ + 1, tt * nt + h : (tt + 1) * nt],
                                in_=yol[:, h:nt],
                            )
                        else:
                            nc.scalar.copy(out=yol[:], in_=ps[:])
                            nc.scalar.dma_start(
                                out=y_d[:, os_ : os_ + 1, tt * nt : (tt + 1) * nt],
                                in_=yol[:],
                            )
                    else:
                        nc.scalar.copy(
                            out=yo[:, os_ * nt : (os_ + 1) * nt], in_=ps[:]
                        )
                if not last:
                    nc.scalar.dma_start(
                        out=y_d[:, :, tt * nt : (tt + 1) * nt], in_=yo[:]
                    )
    nc.compile()
    return nc


def shard_inputs(x, qweight, qzeros, scales, no=NO, t=TP_T):
    """Host-side sharding + bf16 cast + the k-interleave/tile layout for x^T."""
    x2 = np.asarray(x, dtype=np.float32).reshape(T_TOTAL, IN_F)
    qweight = np.ascontiguousarray(np.asarray(qweight, dtype=np.int32))
    qzeros = np.ascontiguousarray(np.asarray(qzeros, dtype=np.int32))
    scales = np.ascontiguousarray(np.asarray(scales, dtype=np.float16))

    xb = x2.astype(ml_dtypes.bfloat16)  # RNE, same as reference astype(bf16)
    xt_shards = []
    for r in range(DP):
        sl = xb[r * t : (r + 1) * t].reshape(NTILE, NT, KB, 128, 8)
        # [tt, col, q, p, j] -> [tt, q, p, j, col]
        xr = np.ascontiguousarray(sl.transpose(0, 2, 3, 4, 1)).reshape(
            NTILE * KB * 128, 8 * NT
        )
        xt_shards.append(xr)

    qw16 = qweight.view(np.int16).reshape(qweight.shape[0], qweight.shape[1], 2)
    in_maps = []
    for core in range(N_CORES):
        r, c = divmod(core, TP)
        qwc = qw16[:, c * no : (c + 1) * no]
        qzc = qzeros[:, c * (no // 8) : (c + 1) * (no // 8)]
        shifts = (np.arange(8, dtype=np.int32) * 4)[None, None, :]
        zc = ((qzc[:, :, None] >> shifts) & 15).astype(np.int16).reshape(
            qzc.shape[0], no
        )
        # block 0 pre-dequantized: row cc*128 + p holds k = 8p + cc.
        # (w_int - z) is exact in fp32; the fp32 product rounds to bf16
        # via RNE exactly like the DVE's mul (fp32 internal, bf16 out).
        scf = scales[:, c * no : (c + 1) * no].astype(np.float32)  # [32, no]
        qb0 = qweight[0:128, c * no : (c + 1) * no]  # [128, no] int32
        u0 = (
            (qb0[:, None, :] >> (4 * np.arange(8, dtype=np.int32))[None, :, None])
            & 15
        ).astype(np.float32)  # [p, cc, no]
        g0 = (8 * np.arange(128)[:, None] + np.arange(8)[None, :]) // GROUP
        deq0 = (u0 - zc.astype(np.float32)[g0]) * scf[g0]  # [p, cc, no] fp32
        wd0 = np.ascontiguousarray(
            deq0.transpose(1, 0, 2).astype(ml_dtypes.bfloat16)
        ).reshape(8 * 128, no)
        in_maps.append(
            {
                "xt": xt_shards[r],
                "qwl": np.ascontiguousarray(qwc[:, :, 0]),
                "qwh": np.ascontiguousarray(qwc[:, :, 1]),
                "zf": np.repeat(zc, 16, axis=0),
                "sc": np.repeat(scales[:, c * no : (c + 1) * no], 16, axis=0),
                "wd0": wd0,
            }
        )
    return in_maps


def assemble_output(results, no=NO, t=TP_T):
    y = np.empty((T_TOTAL, OUT_F), dtype=np.float32)
    for core in range(N_CORES):
        r, c = divmod(core, TP)
        yp = np.asarray(results[core]["y"])  # [128, n_os, t] bf16
        ypart = yp.transpose(1, 0, 2).reshape(no, t)
        y[r * t : (r + 1) * t, c * no : (c + 1) * no] = ypart.T.astype(np.float32)
    return y.reshape(B, S, OUT_F)


_NC_CACHE = {}


def run(x, qweight, qzeros, scales, trace=False, tmpdir=None):
    from concourse.bass_utils import run_bass_kernel_spmd

    if "nc" not in _NC_CACHE:
        _NC_CACHE["nc"] = build_nc()
    nc = _NC_CACHE["nc"]
    in_maps = shard_inputs(x, qweight, qzeros, scales)
    res = run_bass_kernel_spmd(
        nc, in_maps, list(range(N_CORES)), trace=trace, tmpdir=tmpdir
    )
    return assemble_output(res.results), res


def kernel(x, qweight, qzeros, scales):
    # Rare transient infra flakes can corrupt a run wholesale (observed
    # once: 1e36-scale garbage). Outputs here are bounded (|y| < ~100),
    # so a magnitude/finiteness check catches that mode; retry if hit.
    for _ in range(3):
        y, _ = run(x, qweight, qzeros, scales)
        if np.isfinite(y).all() and np.abs(y).max() < 1e6:
            return y
    return y


# revision 25
# speedup vs baseline: 1.0007x; 1.0007x over previous
"""AutoRound/GPTQ int4 linear on 8 Trainium2 NeuronCores.

y = x @ dequant(qweight, qzeros, scales), computed in bf16 like the torch
module: deq = (w_int4 - zeros[g]) * scales[g] in fp32, cast to bf16;
y = bf16_matmul(x.bf16, deq.bf16) with fp32 accumulation, output cast
back to fp32.

Sharding: 8 cores = 4-way tensor-parallel on out_features (1024 each)
x 2-way data-parallel on tokens (4096 each). Each core dequantizes its
weight slice on-chip and computes [1024 out, 4096 tok] bf16; the host
reassembles.

Per-core schedule (PE roofline here is 2048 matmuls x 216 ns = 442 us):
- x is cast to bf16 on the host (RNE, identical to the reference's
  astype) and fed through plain HWDGE DMAs on the sync ring; the old
  SWDGE fp32->bf16 converting-DMA path kept gpsimd busy ~172 us and
  doubled x HBM traffic.
- The contraction (in_features) index is interleaved so that SBUF
  k-chunk `cc = blk*8 + j` holds k = blk*1024 + 8*p + j at partition p.
  Nibble j of packed qweight row p is the weight for partition p of
  chunk cc, so the int4 unpack is one fused shift+mask tensor_scalar
  per chunk with a *constant* shift; qweight is host-split into int16
  low/high planes so the dequant chain runs in 16-bit DVE fast modes.
  zeros/scales are host-unpacked/replicated x16 (tiny metadata).
- Token tile 0 runs chunk-OUTER across all 8 PSUM banks: each dequanted
  chunk is consumed by 8 matmuls (1.7 us) while the DVE produces the
  next chunk in 0.8 us, so the PE never waits for the dequant frontier
  (the old os-outer order stalled ~11 us and re-tripped the HAM
  clock-gate to half rate for 10 us). Tiles 1..7 run os-outer with one
  PSUM bank open at a time.
- All PSUM->SBUF copies run on the SCALAR engine: DVE CAST reads of
  PSUM were observed to stall concurrent matmuls ~432 ns each.
- Block-0 weight DMAs go out in parallel on sync/scalar/gpsimd queues;
  tile-0 x arrives as 32 per-chunk DMAs interleaved with the qwl block
  loads so the first matmul can start ~6.5 us in. A short N=128 dummy-
  matmul warmup fills the preamble window and trips the HAM clock-gate
  to 2.4 GHz before the real stream starts.
- y is staged per token tile in one [128, 8*512] SBUF tile and written
  with a single strided DMA per tile (the last tile uses per-os writes
  to keep the tail short); fewer DMA completions also shorten the
  epilogue semaphore drain.
"""

import numpy as np
import ml_dtypes

PACK = 8
IN_F = 4096
OUT_F = 4096
GROUP = 128
B, S = 4, 2048
T_TOTAL = B * S  # 8192

N_CORES = 8
TP = 4  # out_feature shards
DP = 2  # token shards
NO = OUT_F // TP  # 1024 out features per core
TP_T = T_TOTAL // DP  # 4096 tokens per core
NT = 512  # token tile (matmul moving free dim / one PSUM bank)
NTILE = TP_T // NT  # 8
KB = IN_F // 1024  # 4 k-blocks of 1024 (8 chunks of 128 each) = x quarters
WARMUP_MM = 46


def build_nc(no=NO, t=TP_T, nt=NT, kblocks=KB):
    import concourse.bacc as bacc
    import concourse.mybir as mybir
    from concourse.tile import TileContext

    dt = mybir.dt
    alu = mybir.AluOpType
    n_chunks = kblocks * 8
    n_os = no // 128
    ntile = t // nt

    nc = bacc.Bacc("TRN2", target_bir_lowering=False, debug=False)

    # x^T, k-interleaved and tiled: row (tt*KB + q)*128 + p, col j*nt + c
    # holds x[token tt*nt + c, k = q*1024 + 8p + j] as bf16.
    xt_d = nc.dram_tensor(
        "xt", [ntile * kblocks * 128, 8 * nt], dt.bfloat16, kind="ExternalInput"
    )
    # (w_int - zero) for blocks 1-3, host-unpacked to exact bf16 (values are
    # integers in [-15, 15]); block blk rows (blk-1)*128 + p, free j*no + o.
    # The device applies the group scales (the fp dequant arithmetic) with
    # one DVE mul per chunk — the 3-op unpack chain (~1.85us/chunk) could
    # not keep ahead of tile-0's chunk consumption (1.73us/chunk).
    wz_d = nc.dram_tensor(
        "wz", [(kblocks - 1) * 128, 8 * no], dt.bfloat16, kind="ExternalInput"
    )
    # scales, group rows pre-replicated x16 (only block 1-3 rows are read)
    sc_d = nc.dram_tensor("sc", [kblocks * 128, no], dt.float16, kind="ExternalInput")
    # block 0 fully pre-dequantized on host (bit-identical bf16): the matmul
    # stream's start then gates on a single 0.25MB DMA instead of a 3-DMA
    # fan-in plus a 2us DVE chain, all of which run at ~1/3 rate this early
    # in the kernel.
    wd0_d = nc.dram_tensor("wd0", [128, 8 * no], dt.bfloat16, kind="ExternalInput")
    # y[p, os, tok]: out feature os*128 + p
    y_d = nc.dram_tensor("y", [128, n_os, t], dt.bfloat16, kind="ExternalOutput")

    with TileContext(nc) as tc:
        with (
            tc.tile_pool(name="wd", bufs=1) as wd_pool,
            tc.tile_pool(name="wz", bufs=1) as wz_pool,
            tc.tile_pool(name="sbc", bufs=2) as sbc_pool,
            tc.tile_pool(name="xq", bufs=2) as xq_pool,
            tc.tile_pool(name="ps", bufs=8, space="PSUM") as ps_pool,
            tc.tile_pool(name="yo", bufs=2) as yo_pool,
            tc.tile_pool(name="yol", bufs=2) as yol_pool,
        ):
            # memset first so the PE warmup can start during DMA issue
            warm = sbc_pool.tile([128, 128], dt.bfloat16, tag="warm")
            nc.vector.memset(warm[:], 0.0)

            # PE warmup: small-N dummy matmuls fill the pre-data window and
            # trip the HAM clock gate before the real stream starts.
            ps_w = ps_pool.tile([128, nt], dt.float32, tag="ps")
            for _ in range(WARMUP_MM):
                nc.tensor.matmul(
                    out=ps_w[:, 0:128],
                    lhsT=warm[:],
                    rhs=warm[:],
                    start=True,
                    stop=True,
                )

            # Stream-critical loads, back-to-back on the sync ring in exact
            # consumption order: pre-dequantized block-0 chunk ranges
            # interleaved with tile-0's quarter-0 x chunk ranges,
            # progressively coarser (many small DMAs throttle on the
            # in-flight cap; one big DMA completes too late for chunk 0).
            xq_tiles = {}
            xb0 = xq_pool.tile([128, 8 * nt], dt.bfloat16, tag="xq0", name="x0q0")
            wd0_sb = wd_pool.tile([128, 8 * no], dt.bfloat16, tag="wd0")
            for j0, j1 in ((0, 1), (1, 2), (2, 4), (4, 8)):
                nc.sync.dma_start(
                    out=wd0_sb[:, j0 * no : j1 * no],
                    in_=wd0_d[:, j0 * no : j1 * no],
                )
                nc.sync.dma_start(
                    out=xb0[:, j0 * nt : j1 * nt],
                    in_=xt_d[0:128, j0 * nt : j1 * nt],
                )
            # quarter 1 rides the sync ring right after the critical chunks
            xb1 = xq_pool.tile([128, 8 * nt], dt.bfloat16, tag="xq1", name="x0q1")
            nc.sync.dma_start(out=xb1[:], in_=xt_d[128:256, :])

            # blocks 1-3: (w-z) halves + scales + tile-0 quarters 2/3 on
            # the scalar ring, ordered by first use
            wz_tiles = [None] * kblocks
            sbc_tiles = [None] * kblocks
            t0 = [xb0, xb1]
            for blk in range(1, kblocks):
                wzb = wz_pool.tile([128, 8 * no], dt.bfloat16, tag=f"wz{blk}")
                nc.scalar.dma_start(
                    out=wzb[:, 0 : 4 * no],
                    in_=wz_d[(blk - 1) * 128 : blk * 128, 0 : 4 * no],
                )
                sbc = sbc_pool.tile([128, no], dt.float16, tag=f"sbc{blk % 2}")
                nc.scalar.dma_start(
                    out=sbc[:], in_=sc_d[blk * 128 : (blk + 1) * 128, :]
                )
                nc.scalar.dma_start(
                    out=wzb[:, 4 * no : 8 * no],
                    in_=wz_d[(blk - 1) * 128 : blk * 128, 4 * no : 8 * no],
                )
                wz_tiles[blk] = wzb
                sbc_tiles[blk] = sbc
                if blk == 1:
                    for q in (2, 3):
                        xb = xq_pool.tile(
                            [128, 8 * nt], dt.bfloat16, tag=f"xq{q}", name=f"x0q{q}"
                        )
                        nc.scalar.dma_start(
                            out=xb[:], in_=xt_d[q * 128 : (q + 1) * 128, :]
                        )
                        t0.append(xb)
            xq_tiles[0] = t0

            def load_x(tt):
                tiles = []
                for q in range(kblocks):
                    xb = xq_pool.tile(
                        [128, 8 * nt], dt.bfloat16, tag=f"xq{q}", name=f"x{tt}q{q}"
                    )
                    nc.sync.dma_start(
                        out=xb[:],
                        in_=xt_d[(tt * kblocks + q) * 128 : (tt * kblocks + q + 1) * 128, :],
                    )
                    tiles.append(xb)
                xq_tiles[tt] = tiles

            # ---- dequantize blocks 1-3: one DVE mul per chunk (~0.8us),
            # comfortably ahead of tile-0's 1.73us/chunk consumption
            wd_tiles = [None] * n_chunks
            for blk in range(1, kblocks):
                for j in range(8):
                    cc = blk * 8 + j
                    wdc = wd_pool.tile([128, no], dt.bfloat16, tag=f"wd{cc}")
                    nc.vector.tensor_mul(
                        out=wdc[:],
                        in0=wz_tiles[blk][:, j * no : (j + 1) * no],
                        in1=sbc_tiles[blk][:],
                    )
                    wd_tiles[cc] = wdc

            load_x(1)

            def lhsT(k, os_):
                if k < 8:
                    return wd0_sb[:, k * no + os_ * 128 : k * no + (os_ + 1) * 128]
                return wd_tiles[k][:, os_ * 128 : (os_ + 1) * 128]

            # ---- token tile 0: chunk-outer so the PE chases the dequant
            # frontier with zero stalls (8 PSUM banks accumulate at once).
            ps_t0 = [
                ps_pool.tile([128, nt], dt.float32, tag="ps", name=f"ps0_{o}")
                for o in range(n_os)
            ]
            for k in range(n_chunks):
                rhs = xq_tiles[0][k // 8][:, (k % 8) * nt : (k % 8 + 1) * nt]
                for os_ in range(n_os):
                    nc.tensor.matmul(
                        out=ps_t0[os_][:],
                        lhsT=lhsT(k, os_),
                        rhs=rhs,
                        start=(k == 0),
                        stop=(k == n_chunks - 1),
                    )
            yo = yo_pool.tile([128, n_os * nt], dt.bfloat16, tag="yo", name="yo0")
            for os_ in range(n_os):
                nc.scalar.copy(out=yo[:, os_ * nt : (os_ + 1) * nt], in_=ps_t0[os_][:])
            nc.scalar.dma_start(out=y_d[:, :, 0:nt], in_=yo[:])

            # ---- token tiles 1..: os-outer, one PSUM bank open at a time
            for tt in range(1, ntile):
                if tt + 1 < ntile:
                    load_x(tt + 1)
                xts = xq_tiles.pop(tt)
                last = tt == ntile - 1
                yo = (
                    None
                    if last
                    else yo_pool.tile(
                        [128, n_os * nt], dt.bfloat16, tag="yo", name=f"yo{tt}"
                    )
                )
                for os_ in range(n_os):
                    ps = ps_pool.tile([128, nt], dt.float32, tag="ps", name="ps")
                    for k in range(n_chunks):
                        nc.tensor.matmul(
                            out=ps[:],
                            lhsT=lhsT(k, os_),
                            rhs=xts[k // 8][:, (k % 8) * nt : (k % 8 + 1) * nt],
                            start=(k == 0),
                            stop=(k == n_chunks - 1),
                        )
                    if last:
                        # per-os write-out keeps the final copy+DMA short;
                        # the very last group splits across scalar+vector
                        # and two DMA rings to halve the tail
                        yol = yol_pool.tile(
                            [128, nt], dt.bfloat16, tag="yol", name="yol"
                        )
                        if os_ == n_os - 1:
                            h = nt // 2
                            nc.scalar.copy(out=yol[:, 0:h], in_=ps[:, 0:h])
                            nc.vector.tensor_copy(out=yol[:, h:nt], in_=ps[:, h:nt])
                            nc.scalar.dma_start(
                                out=y_d[:, os_ : os_ + 1, tt * nt : tt * nt + h],
                                in_=yol[:, 0:h],
                            )
                            nc.sync.dma_start(
                                out=y_d[:, os_ : os_ + 1, tt * nt + h : (tt + 1) * nt],
                                in_=yol[:, h:nt],
                            )
                        else:
                            nc.scalar.copy(out=yol[:], in_=ps[:])
                            nc.scalar.dma_start(
                                out=y_d[:, os_ : os_ + 1, tt * nt : (tt + 1) * nt],
                                in_=yol[:],
                            )
                    else:
                        nc.scalar.copy(
                            out=yo[:, os_ * nt : (os_ + 1) * nt], in_=ps[:]
                        )
                if not last:
                    nc.scalar.dma_start(
                        out=y_d[:, :, tt * nt : (tt + 1) * nt], in_=yo[:]
                    )
    nc.compile()
    return nc


def shard_inputs(x, qweight, qzeros, scales, no=NO, t=TP_T):
    """Host-side sharding + bf16 cast + the k-interleave/tile layout for x^T."""
    x2 = np.asarray(x, dtype=np.float32).reshape(T_TOTAL, IN_F)
    qweight = np.ascontiguousarray(np.asarray(qweight, dtype=np.int32))
    qzeros = np.ascontiguousarray(np.asarray(qzeros, dtype=np.int32))
    scales = np.ascontiguousarray(np.asarray(scales, dtype=np.float16))

    xb = x2.astype(ml_dtypes.bfloat16)  # RNE, same as reference astype(bf16)
    xt_shards = []
    for r in range(DP):
        sl = xb[r * t : (r + 1) * t].reshape(NTILE, NT, KB, 128, 8)
        # [tt, col, q, p, j] -> [tt, q, p, j, col]
        xr = np.ascontiguousarray(sl.transpose(0, 2, 3, 4, 1)).reshape(
            NTILE * KB * 128, 8 * NT
        )
        xt_shards.append(xr)

    in_maps = []
    shifts = (np.arange(8, dtype=np.int32) * 4)[None, None, :]
    jj = np.arange(8, dtype=np.int64)[None, :]
    pp = np.arange(128, dtype=np.int64)[:, None]
    for core in range(N_CORES):
        r, c = divmod(core, TP)
        qzc = qzeros[:, c * (no // 8) : (c + 1) * (no // 8)]
        zc = ((qzc[:, :, None] >> shifts) & 15).astype(np.float32).reshape(
            qzc.shape[0], no
        )  # [32 groups, no]
        scf = scales[:, c * no : (c + 1) * no].astype(np.float32)  # [32, no]
        # per block: partition p, free j*no + o holds k = blk*1024 + 8p + j.
        # (w_int - z) is exact in bf16 (ints in [-15,15]); block-0's fp32
        # product rounds to bf16 via RNE exactly like the DVE mul would.
        wd0 = None
        wzs = []
        for blk in range(KB):
            qb = qweight[blk * 128 : (blk + 1) * 128, c * no : (c + 1) * no]
            u = (
                (qb[:, None, :] >> (4 * np.arange(8, dtype=np.int32))[None, :, None])
                & 15
            ).astype(np.float32)  # [p, j, no]
            g = (blk * 1024 + 8 * pp + jj) // GROUP  # [p, j]
            wmz = u - zc[g]  # [p, j, no] fp32, exact small ints
            if blk == 0:
                wd0 = np.ascontiguousarray(
                    (wmz * scf[g]).astype(ml_dtypes.bfloat16).reshape(128, 8 * no)
                )
            else:
                wzs.append(wmz.astype(ml_dtypes.bfloat16).reshape(128, 8 * no))
        in_maps.append(
            {
                "xt": xt_shards[r],
                "wz": np.ascontiguousarray(np.concatenate(wzs, axis=0)),
                "sc": np.repeat(scales[:, c * no : (c + 1) * no], 16, axis=0),
                "wd0": wd0,
            }
        )
    return in_maps


def assemble_output(results, no=NO, t=TP_T):
    y = np.empty((T_TOTAL, OUT_F), dtype=np.float32)
    for core in range(N_CORES):
        r, c = divmod(core, TP)
        yp = np.asarray(results[core]["y"])  # [128, n_os, t] bf16
        ypart = yp.transpose(1, 0, 2).reshape(no, t)
        y[r * t : (r + 1) * t, c * no : (c + 1) * no] = ypart.T.astype(np.float32)
    return y.reshape(B, S, OUT_F)


_NC_CACHE = {}


def run(x, qweight, qzeros, scales, trace=False, tmpdir=None):
    from concourse.bass_utils import run_bass_kernel_spmd

    if "nc" not in _NC_CACHE:
        _NC_CACHE["nc"] = build_nc()
    nc = _NC_CACHE["nc"]
    in_maps = shard_inputs(x, qweight, qzeros, scales)
    res = run_bass_kernel_spmd(
        nc, in_maps, list(range(N_CORES)), trace=trace, tmpdir=tmpdir
    )
    return assemble_output(res.results), res


def kernel(x, qweight, qzeros, scales):
    # Rare transient infra flakes can corrupt a run wholesale (observed
    # once: 1e36-scale garbage). Outputs here are bounded (|y| < ~100),
    # so a magnitude/finiteness check catches that mode; retry if hit.
    for _ in range(3):
        y, _ = run(x, qweight, qzeros, scales)
        if np.isfinite(y).all() and np.abs(y).max() < 1e6:
            return y
    return y


# revision 27
# speedup vs baseline: 1.0197x; 1.0190x over previous
"""AutoRound/GPTQ int4 linear on 8 Trainium2 NeuronCores.

y = x @ dequant(qweight, qzeros, scales), computed in bf16 like the torch
module: deq = (w_int4 - zeros[g]) * scales[g] in fp32, cast to bf16;
y = bf16_matmul(x.bf16, deq.bf16) with fp32 accumulation, output cast
back to fp32.

Sharding: 8 cores = 4-way tensor-parallel on out_features (1024 each)
x 2-way data-parallel on tokens (4096 each). Each core dequantizes its
weight slice on-chip and computes [1024 out, 4096 tok] bf16; the host
reassembles.

Per-core schedule (PE roofline here is 2048 matmuls x 216 ns = 442 us;
steady state runs exactly at that rate):
- x is cast to bf16 on the host (RNE, identical to the reference's
  astype) and fed through plain HWDGE DMAs; the old SWDGE fp32->bf16
  converting-DMA path kept gpsimd busy ~172 us and doubled x traffic.
- The contraction (in_features) index is interleaved so that SBUF
  k-chunk `cc = blk*8 + j` holds k = blk*1024 + 8*p + j at partition p.
  Nibble j of packed qweight row p is the weight for partition p of
  chunk cc, so the int4 unpack is one fused shift+mask tensor_scalar
  per chunk with a *constant* shift; qweight is host-split into int16
  low/high planes so the dequant chain runs in 16-bit DVE fast modes
  (~1.3 us/chunk pipelined). zeros/scales are host-unpacked/replicated.
- Token tile 0 runs chunk-OUTER across all 8 PSUM banks: each dequanted
  chunk is consumed by 8 matmuls (1.73 us) while the DVE produces the
  next in ~1.3 us, so the PE chases the dequant frontier without the
  ~11 us of stalls the os-outer order caused. Tiles 1..7 run os-outer
  with one PSUM bank open at a time.
- All PSUM->SBUF copies run on the SCALAR engine (ACTIVATE); y is
  staged per token tile in one [128, 8*512] SBUF tile and written with
  a single strided DMA (the last tile writes per-os, final group split
  across scalar+vector and both rings, to keep the tail short).
- Startup: DMAs run ~3x below line rate for the first ~15 us and the
  framework preamble is ~7-10 us, so the first real matmul lands
  ~16-19 us in. Critical block-0 loads are spread across both HWDGE
  rings (qwl0+zf0 on sync, sc0 first on scalar), tile-0's x arrives as
  per-chunk DMAs interleaved with the qwl block loads, and ~78 N=128
  dummy matmuls bridge the PE from the preamble to the stream so the
  HAM clock-gate stays at 2.4 GHz instead of re-throttling.
"""

import numpy as np
import ml_dtypes

PACK = 8
IN_F = 4096
OUT_F = 4096
GROUP = 128
B, S = 4, 2048
T_TOTAL = B * S  # 8192

N_CORES = 8
TP = 4  # out_feature shards
DP = 2  # token shards
NO = OUT_F // TP  # 1024 out features per core
TP_T = T_TOTAL // DP  # 4096 tokens per core
NT = 512  # token tile (matmul moving free dim / one PSUM bank)
NTILE = TP_T // NT  # 8
KB = IN_F // 1024  # 4 k-blocks of 1024 (8 chunks of 128 each) = x quarters
WARMUP_MM = 78


def build_nc(no=NO, t=TP_T, nt=NT, kblocks=KB):
    import concourse.bacc as bacc
    import concourse.mybir as mybir
    from concourse.tile import TileContext

    dt = mybir.dt
    alu = mybir.AluOpType
    n_chunks = kblocks * 8
    n_os = no // 128
    ntile = t // nt

    nc = bacc.Bacc("TRN2", target_bir_lowering=False, debug=False)

    # x^T, k-interleaved and tiled: row (tt*KB + q)*128 + p, col j*nt + c
    # holds x[token tt*nt + c, k = q*1024 + 8p + j] as bf16.
    xt_d = nc.dram_tensor(
        "xt", [ntile * kblocks * 128, 8 * nt], dt.bfloat16, kind="ExternalInput"
    )
    # low/high int16 halves of the packed int32 qweight (host-split):
    # nibbles j=0..3 live in the low half, j=4..7 in the high half.
    qwl_d = nc.dram_tensor("qwl", [kblocks * 128, no], dt.int16, kind="ExternalInput")
    qwh_d = nc.dram_tensor("qwh", [kblocks * 128, no], dt.int16, kind="ExternalInput")
    # zeros (host-unpacked int16) and scales, group rows pre-replicated x16
    zf_d = nc.dram_tensor("zf", [kblocks * 128, no], dt.int16, kind="ExternalInput")
    sc_d = nc.dram_tensor("sc", [kblocks * 128, no], dt.float16, kind="ExternalInput")
    # y[p, os, tok]: out feature os*128 + p
    y_d = nc.dram_tensor("y", [128, n_os, t], dt.bfloat16, kind="ExternalOutput")

    with TileContext(nc) as tc:
        with (
            tc.tile_pool(name="wd", bufs=1) as wd_pool,
            tc.tile_pool(name="qw", bufs=2) as qw_pool,
            tc.tile_pool(name="sbc", bufs=2) as sbc_pool,
            tc.tile_pool(name="zf", bufs=2) as zf_pool,
            tc.tile_pool(name="wi", bufs=5) as wi_pool,
            tc.tile_pool(name="xq", bufs=2) as xq_pool,
            tc.tile_pool(name="ps", bufs=8, space="PSUM") as ps_pool,
            tc.tile_pool(name="yo", bufs=2) as yo_pool,
            tc.tile_pool(name="yol", bufs=4) as yol_pool,
        ):
            # memset first so the PE warmup can start during DMA issue
            warm = qw_pool.tile([128, 128], dt.bfloat16, tag="warm")
            nc.vector.memset(warm[:], 0.0)

            qw_sbs = []
            zf_tiles = [None] * kblocks
            sbc_tiles = [None] * kblocks

            def load_block(blk):
                qwl_sb = qw_pool.tile([128, no], dt.int16, tag=f"qwl{blk % 2}")
                qwh_sb = qw_pool.tile([128, no], dt.int16, tag=f"qwh{blk % 2}")
                qw_sbs.append((qwl_sb, qwh_sb))
                zf = zf_pool.tile(
                    [128, no], dt.int16, tag=f"zf{blk % 2}", name=f"zf{blk}"
                )
                sbc = sbc_pool.tile([128, no], dt.float16, tag=f"sbc{blk % 2}")
                nc.sync.dma_start(
                    out=qwl_sb[:], in_=qwl_d[blk * 128 : (blk + 1) * 128, :]
                )
                if blk == 0:
                    # block 0 gates the first dequant chunk and hence the
                    # whole matmul stream. Early DMAs run ~3x below line
                    # rate, so split the three critical tensors across both
                    # HWDGE rings: qwl0+zf0 on sync, sc0 FIRST on scalar
                    # (qwh0 is not needed until chunk 4, ~7us later).
                    nc.sync.dma_start(
                        out=zf[:], in_=zf_d[blk * 128 : (blk + 1) * 128, :]
                    )
                    nc.scalar.dma_start(
                        out=sbc[:], in_=sc_d[blk * 128 : (blk + 1) * 128, :]
                    )
                    nc.scalar.dma_start(
                        out=qwh_sb[:], in_=qwh_d[blk * 128 : (blk + 1) * 128, :]
                    )
                else:
                    nc.scalar.dma_start(
                        out=qwh_sb[:], in_=qwh_d[blk * 128 : (blk + 1) * 128, :]
                    )
                    nc.gpsimd.dma_start(
                        out=zf[:], in_=zf_d[blk * 128 : (blk + 1) * 128, :]
                    )
                    nc.gpsimd.dma_start(
                        out=sbc[:], in_=sc_d[blk * 128 : (blk + 1) * 128, :]
                    )
                zf_tiles[blk] = zf
                sbc_tiles[blk] = sbc

            load_block(0)

            # PE warmup: small-N dummy matmuls bridge the ~9-17us window
            # between the preamble and the first data-ready matmul so the
            # HAM clock gate is warm when the real stream starts.
            ps_w = ps_pool.tile([128, nt], dt.float32, tag="ps")
            for _ in range(WARMUP_MM):
                nc.tensor.matmul(
                    out=ps_w[:, 0:128],
                    lhsT=warm[:],
                    rhs=warm[:],
                    start=True,
                    stop=True,
                )

            # tile-0 x: per-chunk DMAs so chunk 0 lands early, interleaved
            # on the sync ring with the remaining qwl block loads.
            xq_tiles = {}
            t0 = []
            for q in range(kblocks):
                xb = xq_pool.tile(
                    [128, 8 * nt], dt.bfloat16, tag=f"xq{q}", name=f"x0q{q}"
                )
                for j in range(8):
                    nc.sync.dma_start(
                        out=xb[:, j * nt : (j + 1) * nt],
                        in_=xt_d[q * 128 : (q + 1) * 128, j * nt : (j + 1) * nt],
                    )
                t0.append(xb)
                if q + 1 < kblocks:
                    load_block(q + 1)
            xq_tiles[0] = t0

            def load_x(tt):
                tiles = []
                for q in range(kblocks):
                    xb = xq_pool.tile(
                        [128, 8 * nt], dt.bfloat16, tag=f"xq{q}", name=f"x{tt}q{q}"
                    )
                    nc.sync.dma_start(
                        out=xb[:],
                        in_=xt_d[(tt * kblocks + q) * 128 : (tt * kblocks + q + 1) * 128, :],
                    )
                    tiles.append(xb)
                xq_tiles[tt] = tiles

            # ---- dequantize weight slice into 32 per-chunk tiles [128, no]
            wd_tiles = [None] * n_chunks
            for blk in range(kblocks):
                qwl_sb, qwh_sb = qw_sbs[blk]
                for j in range(8):
                    cc = blk * 8 + j
                    wi = wi_pool.tile([128, no], dt.int16, tag="wi_i")
                    nc.vector.tensor_scalar(
                        out=wi[:],
                        in0=(qwl_sb if j < 4 else qwh_sb)[:],
                        scalar1=4 * (j % 4),
                        scalar2=15,
                        op0=alu.logical_shift_right,
                        op1=alu.bitwise_and,
                    )
                    wb = wi_pool.tile([128, no], dt.bfloat16, tag="wi_b")
                    nc.vector.tensor_sub(out=wb[:], in0=wi[:], in1=zf_tiles[blk][:])
                    wdc = wd_pool.tile([128, no], dt.bfloat16, tag=f"wd{cc}")
                    nc.vector.tensor_mul(out=wdc[:], in0=wb[:], in1=sbc_tiles[blk][:])
                    wd_tiles[cc] = wdc

            load_x(1)

            # ---- token tile 0: chunk-outer so the PE chases the dequant
            # frontier with zero stalls (8 PSUM banks accumulate at once).
            ps_t0 = [
                ps_pool.tile([128, nt], dt.float32, tag="ps", name=f"ps0_{o}")
                for o in range(n_os)
            ]
            for k in range(n_chunks):
                rhs = xq_tiles[0][k // 8][:, (k % 8) * nt : (k % 8 + 1) * nt]
                for os_ in range(n_os):
                    nc.tensor.matmul(
                        out=ps_t0[os_][:],
                        lhsT=wd_tiles[k][:, os_ * 128 : (os_ + 1) * 128],
                        rhs=rhs,
                        start=(k == 0),
                        stop=(k == n_chunks - 1),
                    )
            yo = yo_pool.tile([128, n_os * nt], dt.bfloat16, tag="yo", name="yo0")
            for os_ in range(n_os):
                nc.scalar.copy(out=yo[:, os_ * nt : (os_ + 1) * nt], in_=ps_t0[os_][:])
            nc.scalar.dma_start(out=y_d[:, :, 0:nt], in_=yo[:])

            # ---- token tiles 1..: os-outer, one PSUM bank open at a time
            for tt in range(1, ntile):
                if tt + 1 < ntile:
                    load_x(tt + 1)
                xts = xq_tiles.pop(tt)
                last = tt == ntile - 1
                yo = (
                    None
                    if last
                    else yo_pool.tile(
                        [128, n_os * nt], dt.bfloat16, tag="yo", name=f"yo{tt}"
                    )
                )
                for os_ in range(n_os):
                    ps = ps_pool.tile([128, nt], dt.float32, tag="ps", name="ps")
                    for k in range(n_chunks):
                        nc.tensor.matmul(
                            out=ps[:],
                            lhsT=wd_tiles[k][:, os_ * 128 : (os_ + 1) * 128],
                            rhs=xts[k // 8][:, (k % 8) * nt : (k % 8 + 1) * nt],
                            start=(k == 0),
                            stop=(k == n_chunks - 1),
                        )
                    if last:
                        # per-os write-out keeps the final copy+DMA short;
                        # the very last group splits across scalar+vector
                        # and two DMA rings to halve the tail
                        yol = yol_pool.tile(
                            [128, nt], dt.bfloat16, tag="yol", name="yol"
                        )
                        if os_ == n_os - 1:
                            h = nt // 2
                            nc.scalar.copy(out=yol[:, 0:h], in_=ps[:, 0:h])
                            nc.vector.tensor_copy(out=yol[:, h:nt], in_=ps[:, h:nt])
                            nc.scalar.dma_start(
                                out=y_d[:, os_ : os_ + 1, tt * nt : tt * nt + h],
                                in_=yol[:, 0:h],
                            )
                            nc.sync.dma_start(
                                out=y_d[:, os_ : os_ + 1, tt * nt + h : (tt + 1) * nt],
                                in_=yol[:, h:nt],
                            )
                        else:
                            nc.scalar.copy(out=yol[:], in_=ps[:])
                            nc.scalar.dma_start(
                                out=y_d[:, os_ : os_ + 1, tt * nt : (tt + 1) * nt],
                                in_=yol[:],
                            )
                    else:
                        nc.scalar.copy(
                            out=yo[:, os_ * nt : (os_ + 1) * nt], in_=ps[:]
                        )
                if not last:
                    nc.scalar.dma_start(
                        out=y_d[:, :, tt * nt : (tt + 1) * nt], in_=yo[:]
                    )
    nc.compile()
    return nc


def shard_inputs(x, qweight, qzeros, scales, no=NO, t=TP_T):
    """Host-side sharding + bf16 cast + the k-interleave/tile layout for x^T."""
    x2 = np.asarray(x, dtype=np.float32).reshape(T_TOTAL, IN_F)
    qweight = np.ascontiguousarray(np.asarray(qweight, dtype=np.int32))
    qzeros = np.ascontiguousarray(np.asarray(qzeros, dtype=np.int32))
    scales = np.ascontiguousarray(np.asarray(scales, dtype=np.float16))

    xb = x2.astype(ml_dtypes.bfloat16)  # RNE, same as reference astype(bf16)
    xt_shards = []
    for r in range(DP):
        sl = xb[r * t : (r + 1) * t].reshape(NTILE, NT, KB, 128, 8)
        # [tt, col, q, p, j] -> [tt, q, p, j, col]
        xr = np.ascontiguousarray(sl.transpose(0, 2, 3, 4, 1)).reshape(
            NTILE * KB * 128, 8 * NT
        )
        xt_shards.append(xr)

    qw16 = qweight.view(np.int16).reshape(qweight.shape[0], qweight.shape[1], 2)
    in_maps = []
    for core in range(N_CORES):
        r, c = divmod(core, TP)
        qwc = qw16[:, c * no : (c + 1) * no]
        qzc = qzeros[:, c * (no // 8) : (c + 1) * (no // 8)]
        shifts = (np.arange(8, dtype=np.int32) * 4)[None, None, :]
        zc = ((qzc[:, :, None] >> shifts) & 15).astype(np.int16).reshape(
            qzc.shape[0], no
        )
        in_maps.append(
            {
                "xt": xt_shards[r],
                "qwl": np.ascontiguousarray(qwc[:, :, 0]),
                "qwh": np.ascontiguousarray(qwc[:, :, 1]),
                "zf": np.repeat(zc, 16, axis=0),
                "sc": np.repeat(scales[:, c * no : (c + 1) * no], 16, axis=0),
            }
        )
    return in_maps


def assemble_output(results, no=NO, t=TP_T):
    y = np.empty((T_TOTAL, OUT_F), dtype=np.float32)
    for core in range(N_CORES):
        r, c = divmod(core, TP)
        yp = np.asarray(results[core]["y"])  # [128, n_os, t] bf16
        ypart = yp.transpose(1, 0, 2).reshape(no, t)
        y[r * t : (r + 1) * t, c * no : (c + 1) * no] = ypart.T.astype(np.float32)
    return y.reshape(B, S, OUT_F)


_NC_CACHE = {}


def run(x, qweight, qzeros, scales, trace=False, tmpdir=None):
    from concourse.bass_utils import run_bass_kernel_spmd

    if "nc" not in _NC_CACHE:
        _NC_CACHE["nc"] = build_nc()
    nc = _NC_CACHE["nc"]
    in_maps = shard_inputs(x, qweight, qzeros, scales)
    res = run_bass_kernel_spmd(
        nc, in_maps, list(range(N_CORES)), trace=trace, tmpdir=tmpdir
    )
    return assemble_output(res.results), res


def kernel(x, qweight, qzeros, scales):
    # Rare transient infra flakes can corrupt a run wholesale (observed
    # once: 1e36-scale garbage). Outputs here are bounded (|y| < ~100),
    # so a magnitude/finiteness check catches that mode; retry if hit.
    for _ in range(3):
        y, _ = run(x, qweight, qzeros, scales)
        if np.isfinite(y).all() and np.abs(y).max() < 1e6:
            return y
    return y
